# revision 23
# baseline (speedup 1.0000x reference)
"""Trainium2 Bass kernel for nn_DenseJungleSubnet (dense MLP jungle).

Reference computation (B=8192, N_IN=N_OUT=1024, N_HID=64):
    state = [x | zeros(1088)]                        # [B, 2112]
    for i in 0..63:
        out = relu(state[:, :1024+i] @ W_i.T + b_i)  # W_i: [1088-i, 1024+i]
        state[:, 1024+i:] += out
    return state[:, -1024:]

Strategy:
  * Data-parallel over 8 cores: each core takes 1024 batch rows.
  * State is kept TRANSPOSED on-chip ([feature, batch]) so matmul outputs
    (feature-on-partition) can be relu+accumulated in place without any
    transposes: out_T[f_tile, batch] = W~_i @ state_T accumulated over K.
  * Variable layer shapes are padded to a uniform [1152, 1152] (9x9 tiles
    of 128): zero weight columns cover the growing fan-in window, and the
    layer-i output is shifted down by i rows (zero rows at the top) so
    every layer writes the same fixed 1152-row accumulator window.
    Padding with zeros + relu(0)=0 makes the extra rows exact no-ops.
  * Matmuls run as float32r (reduced-precision fp32, 4x faster than fp32
    on the PE array) streaming N=512 batch columns per instruction; the
    accumulator state is kept in full fp32 and only the 128-feature slice
    that is both read and written (state cols 1024..1151) is re-rounded
    to float32r once per layer.
"""

import os
import numpy as np

N_IN = 1024
N_OUT = 1024
N_HID = 64
BATCH = 8192
N_CORES = 8
B_CORE = BATCH // N_CORES   # 1024
PW = 1152                   # padded fan_in / fan_out width (9 * 128)
KT = PW // 128              # 9 contraction tiles
FT = PW // 128              # 9 output-feature tiles
NCHUNK = 512                # moving-operand columns per matmul
NB = B_CORE // NCHUNK       # 2 batch chunks

# knobs (env-overridable for experiments; defaults are the shipped config)
MM_DT_NAME = os.environ.get("BASS_MM_DT", "float32r")
W_DT_NAME = os.environ.get("BASS_W_DT", MM_DT_NAME)
N_LAYERS = int(os.environ.get("BASS_N_LAYERS", str(N_HID)))
TRACE = os.environ.get("BASS_KERNEL_TRACE", "0") == "1"

LAST_EXEC_TIME_NS = None
LAST_RESULTS = None


def _build(mm_dt_name, w_dt_name, n_layers):
    import concourse.bacc as bacc
    import concourse.mybir as mybir
    from concourse.tile import TileContext

    mm_dt = getattr(mybir.dt, mm_dt_name)
    w_dt = getattr(mybir.dt, w_dt_name)
    f32 = mybir.dt.float32
    relu = mybir.ActivationFunctionType.Relu

    nc = bacc.Bacc(None, target_bir_lowering=False)
    xt = nc.dram_tensor("xt", [KT, 128, B_CORE], mm_dt, kind="ExternalInput")
    w = nc.dram_tensor("w", [n_layers, 128, KT * PW], w_dt, kind="ExternalInput")
    bt = nc.dram_tensor("bt", [128, n_layers * FT], f32, kind="ExternalInput")
    out = nc.dram_tensor("out", [FT * 128, B_CORE], f32, kind="ExternalOutput")

    with TileContext(nc) as tc:
        with tc.tile_pool(name="state", bufs=1) as state_pool, \
             tc.tile_pool(name="wpool", bufs=2) as wpool, \
             tc.tile_pool(name="tmp", bufs=6) as tmp_pool, \
             tc.tile_pool(name="psum", bufs=6, space="PSUM") as psum_pool:

            bias = state_pool.tile([128, n_layers * FT], f32, tag="bias")
            nc.sync.dma_start(out=bias[:, :], in_=bt[:, :])

            # K-window (matmul rhs), mm_dt: tiles 0..7 = x^T, tile 8 = shadow
            # of the first accumulator tile (state cols 1024..1151).
            shadow = []
            for t in range(KT):
                s = state_pool.tile([128, B_CORE], mm_dt, tag=f"sh{t}")
                nc.sync.dma_start(out=s[:, :], in_=xt[t])
                shadow.append(s)

            # fp32 master accumulator: state cols 1024..2175 transposed.
            acc = []
            for t in range(FT):
                a = state_pool.tile([128, B_CORE], f32, tag=f"acc{t}")
                nc.vector.memset(a[:, :], 0.0)
                acc.append(a)

            for i in range(n_layers):
                wt = wpool.tile([128, KT * PW], w_dt, tag="w")
                if i == 0:
                    # chunk the first layer's weight load so the PE can start
                    # as soon as the first K-tiles (and x) have landed
                    for kt in range(KT):
                        nc.sync.dma_start(
                            out=wt[:, kt * PW:(kt + 1) * PW],
                            in_=w[i][:, kt * PW:(kt + 1) * PW],
                        )
                else:
                    nc.sync.dma_start(out=wt[:, :], in_=w[i])
                if i > 0:
                    # refresh the read-shadow of acc[0] (rounds fp32 -> mm_dt)
                    nc.scalar.copy(out=shadow[8][:, :], in_=acc[0][:, :])
                # The last output tile (ft=8) has only 64 live features per
                # layer, so consecutive layers' ft=8 blocks are merged into
                # one full 128-row group carried by the odd layer (halves
                # land in acc[8] partitions 0:64 / 64:128; the host sums the
                # halves).  Zero weight-padding makes the newer state
                # snapshot safe for the earlier layer of the pair.
                emit_ft8 = (i % 2 == 1) or (i == n_layers - 1 and n_layers % 2 == 1)
                fts = list(range(FT - 1)) + ([FT - 1] if emit_ft8 else [])
                for b in range(NB):
                    bsl = slice(b * NCHUNK, (b + 1) * NCHUNK)
                    for ft in fts:
                        ps = psum_pool.tile([128, NCHUNK], f32, tag="ps")
                        for kt in range(KT):
                            nc.tensor.matmul(
                                ps[:, :],
                                wt[:, kt * PW + ft * 128: kt * PW + (ft + 1) * 128],
                                shadow[kt][:, bsl],
                                start=(kt == 0),
                                stop=(kt == KT - 1),
                            )
                        tm = tmp_pool.tile([128, NCHUNK], f32, tag="tmp")
                        nc.scalar.activation(
                            tm[:, :], ps[:, :], relu,
                            bias=bias[:, i * FT + ft: i * FT + ft + 1],
                        )
                        nc.vector.tensor_add(
                            out=acc[ft][:, bsl], in0=acc[ft][:, bsl], in1=tm[:, :],
                        )

            for t in range(FT):
                nc.sync.dma_start(out=out[t * 128:(t + 1) * 128, :], in_=acc[t][:, :])

    nc.compile()
    return nc


def _prep_weights(weights, biases, n_layers):
    """Pad each layer to lhsT layout [128(p), KT, PW] per layer, plus the
    per-partition bias table [128, n_layers*FT].  The ft=8 block of each
    odd layer carries the MERGED (i-1, i) pair: features 1024..1087 of
    layer i-1 in rows 1024..1087 and of layer i in rows 1088..1151."""
    wt = np.zeros((n_layers, 1152, KT, 128), dtype=np.float32)  # [i, f, kt, p]
    bt = np.zeros((n_layers, FT, 128), dtype=np.float32)        # [i, ft, p]
    wfull = np.zeros((n_layers, 1152, PW), dtype=np.float32)    # padded W~ [f, k]
    bfull = np.zeros((n_layers, 1152), dtype=np.float32)
    for i in range(n_layers):
        wi = np.asarray(weights[i], dtype=np.float32)   # [fo, fi]
        bi = np.asarray(biases[i], dtype=np.float32)    # [fo]
        fo, fi = wi.shape
        # W~[f, k] = W_i[f - i, k] for i <= f < i+fo, k < fi
        wfull[i, i:i + fo, :fi] = wi
        bfull[i, i:i + fo] = bi
    for i in range(n_layers):
        # ft 0..7 from this layer; ft=8 merged on carrier layers
        emit = wfull[i].copy()
        bemit = bfull[i].copy()
        emit[1024:] = 0.0
        bemit[1024:] = 0.0
        if i % 2 == 1:
            emit[1024:1088] = wfull[i - 1, 1024:1088]
            emit[1088:1152] = wfull[i, 1024:1088]
            bemit[1024:1088] = bfull[i - 1, 1024:1088]
            bemit[1088:1152] = bfull[i, 1024:1088]
        elif i == n_layers - 1:  # unpaired trailing even layer (debug sizes)
            emit[1024:1088] = wfull[i, 1024:1088]
            bemit[1024:1088] = bfull[i, 1024:1088]
        wt[i] = emit.reshape(1152, KT, 128)
        bt[i] = bemit.reshape(FT, 128)
    # SBUF weight layout: partition p, free index kt*PW + f
    wt = np.ascontiguousarray(wt.transpose(0, 3, 2, 1)).reshape(n_layers, 128, KT * PW)
    if W_DT_NAME == "bfloat16":
        import ml_dtypes
        wt = wt.astype(ml_dtypes.bfloat16)
    # bias: [p, i*FT + ft]
    bt = np.ascontiguousarray(bt.transpose(2, 0, 1)).reshape(128, n_layers * FT)
    return wt, bt


def kernel(x, weights, biases):
    global LAST_EXEC_TIME_NS, LAST_RESULTS
    from concourse.bass_utils import run_bass_kernel_spmd

    x = np.asarray(x, dtype=np.float32)
    n_layers = N_LAYERS
    wt, bt = _prep_weights(weights, biases, n_layers)

    nc = _build(MM_DT_NAME, W_DT_NAME, n_layers)

    in_maps = []
    for c in range(N_CORES):
        xc = x[c * B_CORE:(c + 1) * B_CORE, :]          # [1024, 1024]
        xtc = np.zeros((KT, 128, B_CORE), dtype=np.float32)
        xtc[:8] = np.ascontiguousarray(xc.T).reshape(8, 128, B_CORE)
        in_maps.append({"xt": xtc, "w": wt, "bt": bt})

    res = run_bass_kernel_spmd(nc, in_maps, list(range(N_CORES)), trace=TRACE)
    LAST_EXEC_TIME_NS = res.exec_time_ns
    LAST_RESULTS = res

    outs = []
    for c in range(N_CORES):
        r = res.results[c]["out"].copy()                # [1152, 1024]
        # ft=8 halves: even-layer sums sit in rows 1024..1087, odd-layer
        # sums in rows 1088..1151 — both belong to output rows 1024..1087.
        r[1024:1088, :] += r[1088:1152, :]
        outs.append(r[64:64 + N_OUT, :].T)              # [1024, 1024]
    return np.ascontiguousarray(np.concatenate(outs, axis=0))


# revision 25
# speedup vs baseline: 1.0026x; 1.0026x over previous
"""Trainium2 Bass kernel for nn_DenseJungleSubnet (dense MLP jungle).

Reference computation (B=8192, N_IN=N_OUT=1024, N_HID=64):
    state = [x | zeros(1088)]                        # [B, 2112]
    for i in 0..63:
        out = relu(state[:, :1024+i] @ W_i.T + b_i)  # W_i: [1088-i, 1024+i]
        state[:, 1024+i:] += out
    return state[:, -1024:]

Strategy:
  * Data-parallel over 8 cores: each core takes 1024 batch rows.
  * State is kept TRANSPOSED on-chip ([feature, batch]) so matmul outputs
    (feature-on-partition) can be relu+accumulated in place without any
    transposes: out_T[f_tile, batch] = W~_i @ state_T accumulated over K.
  * Variable layer shapes are padded to a uniform [1152, 1152] (9x9 tiles
    of 128): zero weight columns cover the growing fan-in window, and the
    layer-i output is shifted down by i rows (zero rows at the top) so
    every layer writes the same fixed 1152-row accumulator window.
    Padding with zeros + relu(0)=0 makes the extra rows exact no-ops.
  * Matmuls run as float32r (reduced-precision fp32, 4x faster than fp32
    on the PE array) streaming N=512 batch columns per instruction; the
    accumulator state is kept in full fp32 and only the 128-feature slice
    that is both read and written (state cols 1024..1151) is re-rounded
    to float32r once per layer.
"""

import os
import numpy as np

N_IN = 1024
N_OUT = 1024
N_HID = 64
BATCH = 8192
N_CORES = 8
B_CORE = BATCH // N_CORES   # 1024
PW = 1152                   # padded fan_in / fan_out width (9 * 128)
KT = PW // 128              # 9 contraction tiles
FT = PW // 128              # 9 output-feature tiles
NCHUNK = 512                # moving-operand columns per matmul
NB = B_CORE // NCHUNK       # 2 batch chunks

# knobs (env-overridable for experiments; defaults are the shipped config)
MM_DT_NAME = os.environ.get("BASS_MM_DT", "float32r")
W_DT_NAME = os.environ.get("BASS_W_DT", MM_DT_NAME)
N_LAYERS = int(os.environ.get("BASS_N_LAYERS", str(N_HID)))
TRACE = os.environ.get("BASS_KERNEL_TRACE", "0") == "1"

LAST_EXEC_TIME_NS = None
LAST_RESULTS = None


def _build(mm_dt_name, w_dt_name, n_layers):
    import concourse.bacc as bacc
    import concourse.mybir as mybir
    from concourse.tile import TileContext

    mm_dt = getattr(mybir.dt, mm_dt_name)
    w_dt = getattr(mybir.dt, w_dt_name)
    f32 = mybir.dt.float32
    relu = mybir.ActivationFunctionType.Relu

    nc = bacc.Bacc(None, target_bir_lowering=False)
    xt = nc.dram_tensor("xt", [KT, 128, B_CORE], mm_dt, kind="ExternalInput")
    w = nc.dram_tensor("w", [n_layers, 128, KT * PW], w_dt, kind="ExternalInput")
    bt = nc.dram_tensor("bt", [128, n_layers * FT], f32, kind="ExternalInput")
    out = nc.dram_tensor("out", [FT * 128, B_CORE], f32, kind="ExternalOutput")

    with TileContext(nc) as tc:
        with tc.tile_pool(name="state", bufs=1) as state_pool, \
             tc.tile_pool(name="wpool", bufs=2) as wpool, \
             tc.tile_pool(name="tmp", bufs=6) as tmp_pool, \
             tc.tile_pool(name="psum", bufs=6, space="PSUM") as psum_pool:

            bias = state_pool.tile([128, n_layers * FT], f32, tag="bias")
            nc.sync.dma_start(out=bias[:, :], in_=bt[:, :])

            # K-window (matmul rhs), mm_dt: tiles 0..7 = x^T, tile 8 = shadow
            # of the first accumulator tile (state cols 1024..1151).  Tile 8
            # is never read before the first refresh overwrites it (layer 0
            # has no live k8 columns), so it needs no load.  x tiles come in
            # per-batch-chunk halves so the b=0 groups can start sooner.
            shadow = []
            for t in range(KT):
                s = state_pool.tile([128, B_CORE], mm_dt, tag=f"sh{t}")
                if t < KT - 1:
                    for b in range(NB):
                        bsl = slice(b * NCHUNK, (b + 1) * NCHUNK)
                        nc.sync.dma_start(out=s[:, bsl], in_=xt[t][:, bsl])
                shadow.append(s)

            # fp32 master accumulator: state cols 1024..2175 transposed.
            acc = []
            for t in range(FT):
                a = state_pool.tile([128, B_CORE], f32, tag=f"acc{t}")
                nc.vector.memset(a[:, :], 0.0)
                acc.append(a)

            for i in range(n_layers):
                wt = wpool.tile([128, KT * PW], w_dt, tag="w")
                if i == 0:
                    # chunk the first layer's weight load so the PE can start
                    # as soon as the first K-tiles (and x) have landed
                    for kt in range(KT):
                        nc.sync.dma_start(
                            out=wt[:, kt * PW:(kt + 1) * PW],
                            in_=w[i][:, kt * PW:(kt + 1) * PW],
                        )
                else:
                    nc.sync.dma_start(out=wt[:, :], in_=w[i])
                if i > 0:
                    # refresh the read-shadow of acc[0] (rounds fp32 -> mm_dt)
                    nc.scalar.copy(out=shadow[8][:, :], in_=acc[0][:, :])
                # The last output tile (ft=8) has only 64 live features per
                # layer, so consecutive layers' ft=8 blocks are merged into
                # one full 128-row group carried by the odd layer (halves
                # land in acc[8] partitions 0:64 / 64:128; the host sums the
                # halves).  Zero weight-padding makes the newer state
                # snapshot safe for the earlier layer of the pair.
                emit_ft8 = (i % 2 == 1) or (i == n_layers - 1 and n_layers % 2 == 1)
                fts = list(range(FT - 1)) + ([FT - 1] if emit_ft8 else [])
                # layer 0 reads no accumulator columns — its k8 block is
                # entirely zero weights, so skip that matmul
                nkt = KT - 1 if i == 0 else KT
                for b in range(NB):
                    bsl = slice(b * NCHUNK, (b + 1) * NCHUNK)
                    for ft in fts:
                        ps = psum_pool.tile([128, NCHUNK], f32, tag="ps")
                        for kt in range(nkt):
                            nc.tensor.matmul(
                                ps[:, :],
                                wt[:, kt * PW + ft * 128: kt * PW + (ft + 1) * 128],
                                shadow[kt][:, bsl],
                                start=(kt == 0),
                                stop=(kt == nkt - 1),
                            )
                        tm = tmp_pool.tile([128, NCHUNK], f32, tag="tmp")
                        nc.scalar.activation(
                            tm[:, :], ps[:, :], relu,
                            bias=bias[:, i * FT + ft: i * FT + ft + 1],
                        )
                        nc.vector.tensor_add(
                            out=acc[ft][:, bsl], in0=acc[ft][:, bsl], in1=tm[:, :],
                        )

            for t in range(FT):
                nc.sync.dma_start(out=out[t * 128:(t + 1) * 128, :], in_=acc[t][:, :])

    nc.compile()
    return nc


def _prep_weights(weights, biases, n_layers):
    """Pad each layer to lhsT layout [128(p), KT, PW] per layer, plus the
    per-partition bias table [128, n_layers*FT].  The ft=8 block of each
    odd layer carries the MERGED (i-1, i) pair: features 1024..1087 of
    layer i-1 in rows 1024..1087 and of layer i in rows 1088..1151."""
    wt = np.zeros((n_layers, 1152, KT, 128), dtype=np.float32)  # [i, f, kt, p]
    bt = np.zeros((n_layers, FT, 128), dtype=np.float32)        # [i, ft, p]
    wfull = np.zeros((n_layers, 1152, PW), dtype=np.float32)    # padded W~ [f, k]
    bfull = np.zeros((n_layers, 1152), dtype=np.float32)
    for i in range(n_layers):
        wi = np.asarray(weights[i], dtype=np.float32)   # [fo, fi]
        bi = np.asarray(biases[i], dtype=np.float32)    # [fo]
        fo, fi = wi.shape
        # W~[f, k] = W_i[f - i, k] for i <= f < i+fo, k < fi
        wfull[i, i:i + fo, :fi] = wi
        bfull[i, i:i + fo] = bi
    for i in range(n_layers):
        # ft 0..7 from this layer; ft=8 merged on carrier layers
        emit = wfull[i].copy()
        bemit = bfull[i].copy()
        emit[1024:] = 0.0
        bemit[1024:] = 0.0
        if i % 2 == 1:
            emit[1024:1088] = wfull[i - 1, 1024:1088]
            emit[1088:1152] = wfull[i, 1024:1088]
            bemit[1024:1088] = bfull[i - 1, 1024:1088]
            bemit[1088:1152] = bfull[i, 1024:1088]
        elif i == n_layers - 1:  # unpaired trailing even layer (debug sizes)
            emit[1024:1088] = wfull[i, 1024:1088]
            bemit[1024:1088] = bfull[i, 1024:1088]
        wt[i] = emit.reshape(1152, KT, 128)
        bt[i] = bemit.reshape(FT, 128)
    # SBUF weight layout: partition p, free index kt*PW + f
    wt = np.ascontiguousarray(wt.transpose(0, 3, 2, 1)).reshape(n_layers, 128, KT * PW)
    if W_DT_NAME == "bfloat16":
        import ml_dtypes
        wt = wt.astype(ml_dtypes.bfloat16)
    # bias: [p, i*FT + ft]
    bt = np.ascontiguousarray(bt.transpose(2, 0, 1)).reshape(128, n_layers * FT)
    return wt, bt


def kernel(x, weights, biases):
    global LAST_EXEC_TIME_NS, LAST_RESULTS
    from concourse.bass_utils import run_bass_kernel_spmd

    x = np.asarray(x, dtype=np.float32)
    n_layers = N_LAYERS
    wt, bt = _prep_weights(weights, biases, n_layers)

    nc = _build(MM_DT_NAME, W_DT_NAME, n_layers)

    in_maps = []
    for c in range(N_CORES):
        xc = x[c * B_CORE:(c + 1) * B_CORE, :]          # [1024, 1024]
        xtc = np.zeros((KT, 128, B_CORE), dtype=np.float32)
        xtc[:8] = np.ascontiguousarray(xc.T).reshape(8, 128, B_CORE)
        in_maps.append({"xt": xtc, "w": wt, "bt": bt})

    res = run_bass_kernel_spmd(nc, in_maps, list(range(N_CORES)), trace=TRACE)
    LAST_EXEC_TIME_NS = res.exec_time_ns
    LAST_RESULTS = res

    outs = []
    for c in range(N_CORES):
        r = res.results[c]["out"].copy()                # [1152, 1024]
        # ft=8 halves: even-layer sums sit in rows 1024..1087, odd-layer
        # sums in rows 1088..1151 — both belong to output rows 1024..1087.
        r[1024:1088, :] += r[1088:1152, :]
        outs.append(r[64:64 + N_OUT, :].T)              # [1024, 1024]
    return np.ascontiguousarray(np.concatenate(outs, axis=0))
